# revision 4
# baseline (speedup 1.0000x reference)
"""Trainium2 Bass kernel for nn_ConvocationV3 (dense_cnn) — v2.

Pipeline per sample (B=32, C=384, H=W=54, K=3):
  value = conv1x1(x, w_v) ; qk = pool3x3(conv1x1(x, w_qk)) = conv1x1(pool3x3(x), w_qk)
  h = gelu(conv1x1(qk, w_kg1)) ; kernels = conv1x1(h, w_kg2)
  kernels -= sigmoid(beta)/9 * sum_taps(kernels)
  out = depthwise3x3(value, kernels)  (per-sample, per-channel kernels)
  y = conv1x1(out, w_proj)

v2: depthwise runs entirely on the vector engine via hand-authored
3-tap FIR custom DVE ops (one pass per kernel row, delay-line reuse of
the v stream) — no PE diag-matmul taps, no shifted-copy planes.

Sharding: data-parallel over batch, 4 samples per core on 8 cores.
"""

import numpy as np
import ml_dtypes

import concourse.bass as bass
import concourse.bacc as bacc
import concourse.mybir as mybir
import concourse.tile as tile
from concourse.bass_utils import run_bass_kernel_spmd

from fir_ops import FIR3_INIT, FIR3_ACC, emit_fir

F32 = mybir.dt.float32
BF16 = mybir.dt.bfloat16
AX = mybir.AxisListType
ALU = mybir.AluOpType
ACTF = mybir.ActivationFunctionType

B_LOC = 4          # samples per core
CT = 3             # channel tiles (384 = 3*128)
P = 128
HW = 2916          # 54*54
PW = 56            # padded width/height
PLANE = PW * PW    # 3136
BIG = 972          # dma/act chunk (18 rows of 54)
CH = 486           # matmul free chunk (9 rows of 54)
DQ = 96

V = 64             # vpad lead (plane starts at V)
VPAD_N = V + PLANE + 64
H0 = 64            # dw lead
DW_N = H0 + PLANE + 8
EXT = (0, 30, 60)  # per-row output head extension (distinct k-latch slots)
# src0 AP start for row i: V + 1 + (i-1)*56 - EXT[i] - 1  -> 8, 34, 60
FIR_A = tuple(V + 1 + (i - 1) * PW - EXT[i] - 1 for i in range(3))
assert FIR_A == (8, 34, 60)


def build_program():
    nc = bacc.Bacc("TRN2", target_bir_lowering=False, debug=False)

    x_d = nc.dram_tensor("x", [B_LOC, CT, P, HW], BF16, kind="ExternalInput").ap()
    wv_d = nc.dram_tensor("wv", [P, CT, 384], BF16, kind="ExternalInput").ap()
    wproj_d = nc.dram_tensor("wproj", [P, CT, 384], BF16, kind="ExternalInput").ap()
    wqk_d = nc.dram_tensor("wqk", [P, CT, 384], F32, kind="ExternalInput").ap()
    wkg1_d = nc.dram_tensor("wkg1", [P, CT, DQ], F32, kind="ExternalInput").ap()
    wkg2e_d = nc.dram_tensor("wkg2e", [DQ + 1, 384], F32, kind="ExternalInput").ap()
    bv_d = nc.dram_tensor("bv", [P, CT], F32, kind="ExternalInput").ap()
    bqk_d = nc.dram_tensor("bqk", [P, CT], F32, kind="ExternalInput").ap()
    bkg1_d = nc.dram_tensor("bkg1", [DQ, 1], F32, kind="ExternalInput").ap()
    bproj_d = nc.dram_tensor("bproj", [P, CT], F32, kind="ExternalInput").ap()
    fac9_d = nc.dram_tensor("fac9", [P, CT], F32, kind="ExternalInput").ap()

    y_d = nc.dram_tensor("y", [B_LOC, CT, P, HW], F32, kind="ExternalOutput").ap()

    with tile.TileContext(nc) as tc:
        with (
            tc.tile_pool(name="const", bufs=1) as cpool,
            tc.tile_pool(name="xch", bufs=4) as xpool,
            tc.tile_pool(name="vpad", bufs=3) as vppool,
            tc.tile_pool(name="dw", bufs=3) as dwpool,
            tc.tile_pool(name="ych", bufs=4) as ypool,
            tc.tile_pool(name="small", bufs=2) as spool,
            tc.tile_pool(name="mm", bufs=3, space="PSUM") as mmpool,
            tc.tile_pool(name="smallps", bufs=2, space="PSUM") as sppool,
        ):
            # ---- constants ----
            wv = cpool.tile([P, CT, 384], BF16, name="wv_sb")
            wproj = cpool.tile([P, CT, 384], BF16, name="wproj_sb")
            wqk = cpool.tile([P, CT, 384], F32, name="wqk_sb")
            wkg1 = cpool.tile([P, CT, DQ], F32, name="wkg1_sb")
            wkg2e = cpool.tile([DQ + 1, 384], F32, name="wkg2e_sb")
            bv = cpool.tile([P, CT], F32, name="bv_sb")
            bqk = cpool.tile([P, CT], F32, name="bqk_sb")
            bkg1 = cpool.tile([DQ, 1], F32, name="bkg1_sb")
            bproj = cpool.tile([P, CT], F32, name="bproj_sb")
            fac9 = cpool.tile([P, CT], F32, name="fac9_sb")
            for t_sb, t_dr in [(wv, wv_d), (wproj, wproj_d), (wqk, wqk_d),
                               (wkg1, wkg1_d), (wkg2e, wkg2e_d), (bv, bv_d),
                               (bqk, bqk_d), (bkg1, bkg1_d), (bproj, bproj_d),
                               (fac9, fac9_d)]:
                nc.sync.dma_start(t_sb[:], t_dr[:])

            for b in range(B_LOC):
                # ---- stage A: x load, pooling stage 1, value conv -> vpad ----
                pool1 = spool.tile([P, CT * 54, 3], F32, name=f"pool1_{b}",
                                   tag="pool1")
                vpad = vppool.tile([P, CT, VPAD_N], BF16, name=f"vpad_{b}",
                                   tag="vpad")
                for ct in range(CT):
                    vpv = vpad[:, ct, V:V + PLANE].rearrange(
                        "p (h w) -> p h w", h=PW)
                    # zero borders (interior is fully overwritten by ACT)
                    nc.gpsimd.memset(vpv[:, 0:1, :], 0.0)
                    nc.gpsimd.memset(vpv[:, PW - 1:PW, :], 0.0)
                    nc.gpsimd.memset(vpv[:, 1:PW - 1, 0:1], 0.0)
                    nc.gpsimd.memset(vpv[:, 1:PW - 1, PW - 1:PW], 0.0)

                xchs = []
                for g in range(3):  # big chunks of 18 rows
                    xch = xpool.tile([P, CT, BIG], BF16, name=f"x_{b}_{g}", tag="xch")
                    xchs.append(xch)
                    nc.sync.dma_start(
                        xch[:], x_d[b, :, :, g * BIG:(g + 1) * BIG].transpose([1, 0, 2]))
                    for kt in range(CT):
                        nc.vector.tensor_reduce(
                            out=pool1[:, kt * 54 + g * 18:kt * 54 + (g + 1) * 18, :],
                            in_=xch[:, kt].rearrange(
                                "p (h wb w) -> p h wb w", wb=3, w=18),
                            axis=AX.X, op=ALU.add)
                # mt-major: finish each output plane early so its FIR chain
                # can start while the other planes are still in the matmul
                for mt in range(CT):
                    for g in range(3):
                        xch = xchs[g]
                        ps = mmpool.tile([P, 2, 512], F32, name=f"vps_{b}_{g}_{mt}", tag="mm")
                        for s in range(2):
                            for kt in range(CT):
                                nc.tensor.matmul(
                                    ps[:, s, :CH],
                                    lhsT=wv[:, kt, mt * P:(mt + 1) * P],
                                    rhs=xch[:, kt, s * CH:(s + 1) * CH],
                                    start=(kt == 0), stop=(kt == CT - 1))
                        # write value (+bias) into padded interior rows, bf16
                        nc.scalar.activation(
                            out=vpad[:, mt, V:V + PLANE].rearrange(
                                "p (h w) -> p h w", h=PW)[
                                :, 1 + g * 18:1 + (g + 1) * 18, 1:55],
                            in_=ps[:, :, :CH],
                            func=ACTF.Identity, bias=bv[:, mt:mt + 1], scale=1.0)

                # ---- stage B: pooling stage 2 -> pooled (sum over 324, /324 in wqk) ----
                pooled = spool.tile([P, CT, 9], F32, name=f"pooled_{b}", tag="pooled")
                for kt in range(CT):
                    nc.vector.tensor_reduce(
                        out=pooled[:, kt].rearrange("p (hb wb) -> p hb wb", hb=3),
                        in_=pool1[:, kt * 54:(kt + 1) * 54, :].rearrange(
                            "p (hb hs) wb -> p hb wb hs", hb=3),
                        axis=AX.X, op=ALU.add)

                # ---- stage C: qk conv (f32r, tiny) ----
                qk = spool.tile([P, CT, 9], F32, name=f"qk_{b}", tag="qk")
                for mt in range(CT):
                    psq = sppool.tile([P, 9], F32, name=f"qps_{b}_{mt}", tag="sps")
                    for kt in range(CT):
                        nc.tensor.matmul(
                            psq[:],
                            lhsT=wqk[:, kt, mt * P:(mt + 1) * P],
                            rhs=pooled[:, kt],
                            start=(kt == 0), stop=(kt == CT - 1))
                    nc.scalar.activation(out=qk[:, mt], in_=psq[:],
                                         func=ACTF.Identity, bias=bqk[:, mt:mt + 1],
                                         scale=1.0)

                # ---- stage D: kg1 + gelu ----
                hsb = spool.tile([DQ + 1, 9], F32, name=f"h_{b}", tag="h")
                psh = sppool.tile([DQ, 9], F32, name=f"hps_{b}", tag="sps")
                for kt in range(CT):
                    nc.tensor.matmul(
                        psh[:],
                        lhsT=wkg1[:, kt, :],
                        rhs=qk[:, kt],
                        start=(kt == 0), stop=(kt == CT - 1))
                nc.scalar.activation(out=hsb[:DQ, :], in_=psh[:], func=ACTF.Gelu,
                                     bias=bkg1[:, 0:1], scale=1.0)
                nc.gpsimd.memset(hsb[DQ:DQ + 1, :], 1.0)  # bias row for kg2

                # ---- stage E: kg2 + mean subtraction -> ksb ----
                ksb = spool.tile([P, CT, 9], F32, name=f"k_{b}", tag="ksb")
                ksum = spool.tile([P, CT], F32, name=f"ksum_{b}", tag="ksum")
                for mt in range(CT):
                    psk = sppool.tile([P, 9], F32, name=f"kps_{b}_{mt}", tag="sps")
                    nc.tensor.matmul(
                        psk[:],
                        lhsT=wkg2e[:, mt * P:(mt + 1) * P],
                        rhs=hsb[:],
                        start=True, stop=True)
                    nc.vector.tensor_reduce(out=ksum[:, mt:mt + 1], in_=psk[:],
                                            axis=AX.X, op=ALU.add)
                    nc.vector.tensor_scalar(
                        out=ksum[:, mt:mt + 1], in0=ksum[:, mt:mt + 1],
                        scalar1=fac9[:, mt:mt + 1], scalar2=None, op0=ALU.mult)
                    nc.vector.tensor_scalar(
                        out=ksb[:, mt], in0=psk[:],
                        scalar1=ksum[:, mt:mt + 1], scalar2=None, op0=ALU.subtract)

                # ---- stage F: k-latch slots into vpad lead cells ----
                for ct in range(CT):
                    for i in range(3):
                        nc.vector.tensor_copy(
                            vpad[:, ct, FIR_A[i]:FIR_A[i] + 1],
                            ksb[:, ct, 3 * i:3 * i + 1])

                # ---- stage G: depthwise via 3-tap FIR row passes ----
                dwt = dwpool.tile([P, CT, DW_N], BF16, name=f"dw_{b}", tag="dw")
                for ct in range(CT):
                    ln0 = EXT[0] + PLANE + 2
                    emit_fir(nc.vector, FIR3_INIT,
                             out=dwt[:, ct, H0 - EXT[0]:H0 - EXT[0] + ln0],
                             in0=vpad[:, ct, FIR_A[0]:FIR_A[0] + ln0 + 1],
                             s0=ksb[:, ct, 2:3], s1=ksb[:, ct, 1:2])
                    for i in (1, 2):
                        ln = EXT[i] + PLANE + 2
                        emit_fir(nc.vector, FIR3_ACC,
                                 out=dwt[:, ct, H0 - EXT[i]:H0 - EXT[i] + ln],
                                 in0=vpad[:, ct, FIR_A[i]:FIR_A[i] + ln + 1],
                                 in1=dwt[:, ct, H0 - EXT[i]:H0 - EXT[i] + ln],
                                 s0=ksb[:, ct, 3 * i + 2:3 * i + 3],
                                 s1=ksb[:, ct, 3 * i + 1:3 * i + 2])

                # ---- stage H: proj conv + bias -> y ----
                for mt in range(CT):
                    for g in range(3):
                        ps = mmpool.tile([P, 2, 512], F32, name=f"pps_{b}_{mt}_{g}",
                                         tag="mm")
                        for s in range(2):
                            ch = g * 2 + s
                            for kt in range(CT):
                                nc.tensor.matmul(
                                    ps[:, s, :CH],
                                    lhsT=wproj[:, kt, mt * P:(mt + 1) * P],
                                    rhs=dwt[:, kt, H0:H0 + PLANE].rearrange(
                                        "p (h w) -> p h w", h=PW)[
                                        :, ch * 9 + 1: ch * 9 + 10, 1:55],
                                    start=(kt == 0), stop=(kt == CT - 1))
                        ych = ypool.tile([P, BIG], F32, name=f"y_{b}_{mt}_{g}",
                                         tag="ych")
                        nc.scalar.activation(out=ych[:], in_=ps[:, :, :CH],
                                             func=ACTF.Identity,
                                             bias=bproj[:, mt:mt + 1], scale=1.0)
                        nc.sync.dma_start(
                            y_d[b, mt, :, g * BIG:(g + 1) * BIG], ych[:])
    nc.compile()
    return nc


def _prep_inputs(x, w_qk, b_qk, w_kg1, b_kg1, w_kg2, b_kg2, w_v, b_v,
                 w_proj, b_proj, beta):
    bf = ml_dtypes.bfloat16
    f32 = np.float32

    def lay_w(w, dt):  # (O, Cin) -> lhsT layout [p, kt, O]
        wt = np.ascontiguousarray(w.T.reshape(CT, P, -1).transpose(1, 0, 2))
        return wt.astype(dt)

    def lay_b(v):  # (C,) -> [p, ct]
        return np.ascontiguousarray(v.reshape(CT, P).T).astype(f32)

    consts = {
        "wv": lay_w(w_v, bf),
        "wproj": lay_w(w_proj, bf),
        "wqk": lay_w(w_qk / 324.0, f32),
        "wkg1": lay_w(w_kg1, f32),
        "wkg2e": np.ascontiguousarray(
            np.vstack([w_kg2.T, b_kg2[None, :]])).astype(f32),
        "bv": lay_b(b_v),
        "bqk": lay_b(b_qk),
        "bkg1": np.ascontiguousarray(b_kg1.reshape(DQ, 1)).astype(f32),
        "bproj": lay_b(b_proj),
        "fac9": lay_b(1.0 / (1.0 + np.exp(-beta.astype(np.float64))) / 9.0),
    }
    xs = np.ascontiguousarray(
        x.reshape(8, B_LOC, CT, P, HW)).astype(bf)
    in_maps = [dict(consts, x=np.ascontiguousarray(xs[c])) for c in range(8)]
    return in_maps


_CACHED_NC = None


def kernel(**inputs):
    global _CACHED_NC
    in_maps = _prep_inputs(**{k: np.asarray(v) for k, v in inputs.items()})
    if _CACHED_NC is None:
        _CACHED_NC = build_program()
    res = run_bass_kernel_spmd(_CACHED_NC, in_maps, core_ids=list(range(8)))
    ys = np.stack([r["y"] for r in res.results])  # (8, 4, 3, 128, 2916)
    return ys.reshape(32, 384, 54, 54).astype(np.float32)


# revision 5
# speedup vs baseline: 1.1948x; 1.1948x over previous
"""Trainium2 Bass kernel for nn_ConvocationV3 (dense_cnn) — v2.

Pipeline per sample (B=32, C=384, H=W=54, K=3):
  value = conv1x1(x, w_v) ; qk = pool3x3(conv1x1(x, w_qk)) = conv1x1(pool3x3(x), w_qk)
  h = gelu(conv1x1(qk, w_kg1)) ; kernels = conv1x1(h, w_kg2)
  kernels -= sigmoid(beta)/9 * sum_taps(kernels)
  out = depthwise3x3(value, kernels)  (per-sample, per-channel kernels)
  y = conv1x1(out, w_proj)

v2: depthwise runs entirely on the vector engine via hand-authored
3-tap FIR custom DVE ops (one pass per kernel row, delay-line reuse of
the v stream) — no PE diag-matmul taps, no shifted-copy planes.

Sharding: data-parallel over batch, 4 samples per core on 8 cores.
"""

import numpy as np
import ml_dtypes

import concourse.bass as bass
import concourse.bacc as bacc
import concourse.mybir as mybir
import concourse.tile as tile
from concourse.bass_utils import run_bass_kernel_spmd

from fir_ops import FIR3_INIT, FIR3_ACC, emit_fir

F32 = mybir.dt.float32
BF16 = mybir.dt.bfloat16
AX = mybir.AxisListType
ALU = mybir.AluOpType
ACTF = mybir.ActivationFunctionType

B_LOC = 4          # samples per core
CT = 3             # channel tiles (384 = 3*128)
P = 128
HW = 2916          # 54*54
PW = 56            # padded width/height
PLANE = PW * PW    # 3136
BIG = 972          # dma/act chunk (18 rows of 54)
CH = 486           # matmul free chunk (9 rows of 54)
DQ = 96

V = 64             # vpad lead (plane starts at V)
VPAD_N = V + PLANE + 64
H0 = 64            # dw lead
DW_N = H0 + PLANE + 8
EXT = (0, 30, 60)  # per-row output head extension (distinct k-latch slots)
# src0 AP start for row i: V + 1 + (i-1)*56 - EXT[i] - 1  -> 8, 34, 60
FIR_A = tuple(V + 1 + (i - 1) * PW - EXT[i] - 1 for i in range(3))
assert FIR_A == (8, 34, 60)


def build_program():
    nc = bacc.Bacc("TRN2", target_bir_lowering=False, debug=False)

    x_d = nc.dram_tensor("x", [B_LOC, CT, P, HW], BF16, kind="ExternalInput").ap()
    wv_d = nc.dram_tensor("wv", [P, CT, 384], BF16, kind="ExternalInput").ap()
    wproj_d = nc.dram_tensor("wproj", [P, CT, 384], BF16, kind="ExternalInput").ap()
    wqk_d = nc.dram_tensor("wqk", [P, CT, 384], F32, kind="ExternalInput").ap()
    wkg1_d = nc.dram_tensor("wkg1", [P, CT, DQ], F32, kind="ExternalInput").ap()
    wkg2e_d = nc.dram_tensor("wkg2e", [DQ + 1, 384], F32, kind="ExternalInput").ap()
    bv_d = nc.dram_tensor("bv", [P, CT], F32, kind="ExternalInput").ap()
    bqk_d = nc.dram_tensor("bqk", [P, CT], F32, kind="ExternalInput").ap()
    bkg1_d = nc.dram_tensor("bkg1", [DQ, 1], F32, kind="ExternalInput").ap()
    bproj_d = nc.dram_tensor("bproj", [P, CT], F32, kind="ExternalInput").ap()
    fac9_d = nc.dram_tensor("fac9", [P, CT], F32, kind="ExternalInput").ap()

    y_d = nc.dram_tensor("y", [B_LOC, CT, P, HW], F32, kind="ExternalOutput").ap()

    with tile.TileContext(nc) as tc:
        with (
            tc.tile_pool(name="const", bufs=1) as cpool,
            tc.tile_pool(name="xch", bufs=7) as xpool,
            tc.tile_pool(name="vpad", bufs=2) as vppool,
            tc.tile_pool(name="dw", bufs=2) as dwpool,
            tc.tile_pool(name="ych", bufs=4) as ypool,
            tc.tile_pool(name="small", bufs=2) as spool,
            tc.tile_pool(name="mm", bufs=3, space="PSUM") as mmpool,
            tc.tile_pool(name="smallps", bufs=2, space="PSUM") as sppool,
        ):
            # ---- constants ----
            wv = cpool.tile([P, CT, 384], BF16, name="wv_sb")
            wproj = cpool.tile([P, CT, 384], BF16, name="wproj_sb")
            wqk = cpool.tile([P, CT, 384], F32, name="wqk_sb")
            wkg1 = cpool.tile([P, CT, DQ], F32, name="wkg1_sb")
            wkg2e = cpool.tile([DQ + 1, 384], F32, name="wkg2e_sb")
            bv = cpool.tile([P, CT], F32, name="bv_sb")
            bqk = cpool.tile([P, CT], F32, name="bqk_sb")
            bkg1 = cpool.tile([DQ, 1], F32, name="bkg1_sb")
            bproj = cpool.tile([P, CT], F32, name="bproj_sb")
            fac9 = cpool.tile([P, CT], F32, name="fac9_sb")
            for t_sb, t_dr in [(wv, wv_d), (wproj, wproj_d), (wqk, wqk_d),
                               (wkg1, wkg1_d), (wkg2e, wkg2e_d), (bv, bv_d),
                               (bqk, bqk_d), (bkg1, bkg1_d), (bproj, bproj_d),
                               (fac9, fac9_d)]:
                nc.sync.dma_start(t_sb[:], t_dr[:])

            xchs_all = {}
            pool1_all = {}
            vpad_all = {}
            dw_all = {}
            ksb_all = {}

            def stage_in(b):
                pool1 = spool.tile([P, CT * 54, 3], F32, name=f"pool1_{b}",
                                   tag="pool1")
                pool1_all[b] = pool1
                xchs = []
                for g in range(3):
                    xch = xpool.tile([P, CT, BIG], BF16, name=f"x_{b}_{g}",
                                     tag="xch")
                    xchs.append(xch)
                    nc.sync.dma_start(
                        xch[:],
                        x_d[b, :, :, g * BIG:(g + 1) * BIG].transpose([1, 0, 2]))
                    for kt in range(CT):
                        nc.vector.tensor_reduce(
                            out=pool1[:, kt * 54 + g * 18:kt * 54 + (g + 1) * 18, :],
                            in_=xch[:, kt].rearrange(
                                "p (h wb w) -> p h wb w", wb=3, w=18),
                            axis=AX.X, op=ALU.add)
                xchs_all[b] = xchs

            def stage_val(b):
                xchs = xchs_all[b]
                pool1 = pool1_all[b]
                vpad = vppool.tile([P, CT, VPAD_N], BF16, name=f"vpad_{b}",
                                   tag="vpad")
                vpad_all[b] = vpad
                for ct in range(CT):
                    vpv = vpad[:, ct, V:V + PLANE].rearrange(
                        "p (h w) -> p h w", h=PW)
                    nc.gpsimd.memset(vpv[:, 0:1, :], 0.0)
                    nc.gpsimd.memset(vpv[:, PW - 1:PW, :], 0.0)
                    nc.gpsimd.memset(vpv[:, 1:PW - 1, 0:1], 0.0)
                    nc.gpsimd.memset(vpv[:, 1:PW - 1, PW - 1:PW], 0.0)
                # mt-major: finish each output plane early
                for mt in range(CT):
                    for g in range(3):
                        xch = xchs[g]
                        ps = mmpool.tile([P, 2, 512], F32,
                                         name=f"vps_{b}_{g}_{mt}", tag="mm")
                        for s in range(2):
                            for kt in range(CT):
                                nc.tensor.matmul(
                                    ps[:, s, :CH],
                                    lhsT=wv[:, kt, mt * P:(mt + 1) * P],
                                    rhs=xch[:, kt, s * CH:(s + 1) * CH],
                                    start=(kt == 0), stop=(kt == CT - 1))
                        nc.scalar.activation(
                            out=vpad[:, mt, V:V + PLANE].rearrange(
                                "p (h w) -> p h w", h=PW)[
                                :, 1 + g * 18:1 + (g + 1) * 18, 1:55],
                            in_=ps[:, :, :CH],
                            func=ACTF.Identity, bias=bv[:, mt:mt + 1], scale=1.0)

                # pooling stage 2
                pooled = spool.tile([P, CT, 9], F32, name=f"pooled_{b}",
                                    tag="pooled")
                for kt in range(CT):
                    nc.vector.tensor_reduce(
                        out=pooled[:, kt].rearrange("p (hb wb) -> p hb wb", hb=3),
                        in_=pool1[:, kt * 54:(kt + 1) * 54, :].rearrange(
                            "p (hb hs) wb -> p hb wb hs", hb=3),
                        axis=AX.X, op=ALU.add)
                # qk conv
                qk = spool.tile([P, CT, 9], F32, name=f"qk_{b}", tag="qk")
                for mt in range(CT):
                    psq = sppool.tile([P, 9], F32, name=f"qps_{b}_{mt}", tag="sps")
                    for kt in range(CT):
                        nc.tensor.matmul(
                            psq[:],
                            lhsT=wqk[:, kt, mt * P:(mt + 1) * P],
                            rhs=pooled[:, kt],
                            start=(kt == 0), stop=(kt == CT - 1))
                    nc.scalar.activation(out=qk[:, mt], in_=psq[:],
                                         func=ACTF.Identity,
                                         bias=bqk[:, mt:mt + 1], scale=1.0)
                # kg1 + gelu
                hsb = spool.tile([DQ + 1, 9], F32, name=f"h_{b}", tag="h")
                psh = sppool.tile([DQ, 9], F32, name=f"hps_{b}", tag="sps")
                for kt in range(CT):
                    nc.tensor.matmul(
                        psh[:],
                        lhsT=wkg1[:, kt, :],
                        rhs=qk[:, kt],
                        start=(kt == 0), stop=(kt == CT - 1))
                nc.scalar.activation(out=hsb[:DQ, :], in_=psh[:], func=ACTF.Gelu,
                                     bias=bkg1[:, 0:1], scale=1.0)
                nc.gpsimd.memset(hsb[DQ:DQ + 1, :], 1.0)
                # kg2 + mean subtraction -> ksb
                ksb = spool.tile([P, CT, 9], F32, name=f"k_{b}", tag="ksb")
                ksum = spool.tile([P, CT], F32, name=f"ksum_{b}", tag="ksum")
                ksb_all[b] = ksb
                for mt in range(CT):
                    psk = sppool.tile([P, 9], F32, name=f"kps_{b}_{mt}", tag="sps")
                    nc.tensor.matmul(
                        psk[:],
                        lhsT=wkg2e[:, mt * P:(mt + 1) * P],
                        rhs=hsb[:],
                        start=True, stop=True)
                    nc.vector.tensor_reduce(out=ksum[:, mt:mt + 1], in_=psk[:],
                                            axis=AX.X, op=ALU.add)
                    nc.vector.tensor_scalar(
                        out=ksum[:, mt:mt + 1], in0=ksum[:, mt:mt + 1],
                        scalar1=fac9[:, mt:mt + 1], scalar2=None, op0=ALU.mult)
                    nc.vector.tensor_scalar(
                        out=ksb[:, mt], in0=psk[:],
                        scalar1=ksum[:, mt:mt + 1], scalar2=None,
                        op0=ALU.subtract)
                # k-latch slots
                for ct in range(CT):
                    for i in range(3):
                        nc.vector.tensor_copy(
                            vpad[:, ct, FIR_A[i]:FIR_A[i] + 1],
                            ksb[:, ct, 3 * i:3 * i + 1])

            def stage_fir(b):
                vpad = vpad_all[b]
                ksb = ksb_all[b]
                dwt = dwpool.tile([P, CT, DW_N], BF16, name=f"dw_{b}", tag="dw")
                dw_all[b] = dwt
                for ct in range(CT):
                    ln0 = EXT[0] + PLANE + 2
                    emit_fir(nc.vector, FIR3_INIT,
                             out=dwt[:, ct, H0 - EXT[0]:H0 - EXT[0] + ln0],
                             in0=vpad[:, ct, FIR_A[0]:FIR_A[0] + ln0 + 1],
                             s0=ksb[:, ct, 2:3], s1=ksb[:, ct, 1:2])
                    for i in (1, 2):
                        ln = EXT[i] + PLANE + 2
                        emit_fir(nc.vector, FIR3_ACC,
                                 out=dwt[:, ct, H0 - EXT[i]:H0 - EXT[i] + ln],
                                 in0=vpad[:, ct, FIR_A[i]:FIR_A[i] + ln + 1],
                                 in1=dwt[:, ct, H0 - EXT[i]:H0 - EXT[i] + ln],
                                 s0=ksb[:, ct, 3 * i + 2:3 * i + 3],
                                 s1=ksb[:, ct, 3 * i + 1:3 * i + 2])

            def stage_proj(b):
                dwt = dw_all[b]
                for mt in range(CT):
                    for g in range(3):
                        ps = mmpool.tile([P, 2, 512], F32,
                                         name=f"pps_{b}_{mt}_{g}", tag="mm")
                        for s in range(2):
                            ch = g * 2 + s
                            for kt in range(CT):
                                nc.tensor.matmul(
                                    ps[:, s, :CH],
                                    lhsT=wproj[:, kt, mt * P:(mt + 1) * P],
                                    rhs=dwt[:, kt, H0:H0 + PLANE].rearrange(
                                        "p (h w) -> p h w", h=PW)[
                                        :, ch * 9 + 1: ch * 9 + 10, 1:55],
                                    start=(kt == 0), stop=(kt == CT - 1))
                        ych = ypool.tile([P, BIG], F32, name=f"y_{b}_{mt}_{g}",
                                         tag="ych")
                        nc.scalar.activation(out=ych[:], in_=ps[:, :, :CH],
                                             func=ACTF.Identity,
                                             bias=bproj[:, mt:mt + 1], scale=1.0)
                        nc.sync.dma_start(
                            y_d[b, mt, :, g * BIG:(g + 1) * BIG], ych[:])

            # software-pipelined emission (per-engine queues run in order):
            # Tensor: val(0) val(1) proj(0) val(2) proj(1) val(3) proj(2) proj(3)
            stage_in(0)
            stage_val(0)
            stage_in(1)
            stage_fir(0)
            stage_val(1)
            stage_in(2)
            stage_proj(0)
            stage_fir(1)
            stage_val(2)
            stage_in(3)
            stage_proj(1)
            stage_fir(2)
            stage_val(3)
            stage_proj(2)
            stage_fir(3)
            stage_proj(3)
    nc.compile()
    return nc


def _prep_inputs(x, w_qk, b_qk, w_kg1, b_kg1, w_kg2, b_kg2, w_v, b_v,
                 w_proj, b_proj, beta):
    bf = ml_dtypes.bfloat16
    f32 = np.float32

    def lay_w(w, dt):  # (O, Cin) -> lhsT layout [p, kt, O]
        wt = np.ascontiguousarray(w.T.reshape(CT, P, -1).transpose(1, 0, 2))
        return wt.astype(dt)

    def lay_b(v):  # (C,) -> [p, ct]
        return np.ascontiguousarray(v.reshape(CT, P).T).astype(f32)

    consts = {
        "wv": lay_w(w_v, bf),
        "wproj": lay_w(w_proj, bf),
        "wqk": lay_w(w_qk / 324.0, f32),
        "wkg1": lay_w(w_kg1, f32),
        "wkg2e": np.ascontiguousarray(
            np.vstack([w_kg2.T, b_kg2[None, :]])).astype(f32),
        "bv": lay_b(b_v),
        "bqk": lay_b(b_qk),
        "bkg1": np.ascontiguousarray(b_kg1.reshape(DQ, 1)).astype(f32),
        "bproj": lay_b(b_proj),
        "fac9": lay_b(1.0 / (1.0 + np.exp(-beta.astype(np.float64))) / 9.0),
    }
    xs = np.ascontiguousarray(
        x.reshape(8, B_LOC, CT, P, HW)).astype(bf)
    in_maps = [dict(consts, x=np.ascontiguousarray(xs[c])) for c in range(8)]
    return in_maps


_CACHED_NC = None


def kernel(**inputs):
    global _CACHED_NC
    in_maps = _prep_inputs(**{k: np.asarray(v) for k, v in inputs.items()})
    if _CACHED_NC is None:
        _CACHED_NC = build_program()
    res = run_bass_kernel_spmd(_CACHED_NC, in_maps, core_ids=list(range(8)))
    ys = np.stack([r["y"] for r in res.results])  # (8, 4, 3, 128, 2916)
    return ys.reshape(32, 384, 54, 54).astype(np.float32)
